# revision 8
# baseline (speedup 1.0000x reference)
"""Trainium2 Bass kernel for nn_DistanceLayer (shapelet min-distance), v2.

reference semantics:
  x: (512, 1, 2048), shapelets: (128, 1, 64)
  patches = sliding windows of x (len 64, stride 1), mean-centered
  out[b, s] = min_p ||patch(b, p) - shapelets[s]||_2          -> (512, 128)

Math (negated domain so the reduction is a MAX):
  With s~ = sh - mean_l(sh):  (w - mean(w)) . sh = w . s~
    d2[b,s,p] = A[b,p] + s2[s] - 2 w.s~,  A = sum(w^2) - (sum w)^2/64
  PE computes  v = 2 w.s~ - A  per (s, window); max_p v = s2 - min_p d2,
  so  out = sqrt(relu(s2 - max_p v)).

v2 drain strategy (HW-measured op costs): PSUM can only exit via ACT
(1 elem/lane/cyc @1.2GHz) or DVE (@0.96GHz), so the 16.3M-value v
matrix is split between two drain paths balanced by an LP:
  'a' (46 r's): two ACT casts PSUM fp32 -> SBUF fp16 (~1us each,
      (172+FD) cyc), then one DVE tensor_tensor max fold (2x_1P on
      fp16 SBUF, ~1.1us per r) into one of two alternating fp16
      accumulators;
  'b' (18 r's): two DVE tensor_tensor max ops straight off PSUM into
      a (j,b)-layout fp16 accumulator (~1.2us each, no ACT).
The v1 'd' path (strided tensor_reduce from PSUM, ~2.9us/r in situ)
and STT folds (TensorScalarPtr is 1x on HW, not the hoped 4x; plain
TT gets 2x_1P) are gone.  ACT ~95us and DVE ~100us busy bracket the
span; PE (~60us) hides underneath.  All accumulator merges are
emitted mid-sweep (r=54/56/62) so the tail after the last cast is
just one reduce + sqrt + PE transpose + store.

Setup: ACT's queue stays clear of DMAs (a dep-stalled ring entry
blocks everything behind it): sync ring carries the XBAR transposes +
CT x-row copies, gpsimd (SWDGE) carries the x16 cast-load + Wn +
consts, scalar carries only the late CT A-row copies.  x16 load,
transposes, squares (on DVE) and the A-term pipeline are chunked
two ways so first mains start after half the x load; the j=31
garbage column was eliminated (odd-half second matmul is N=448), so
no garbage memsets exist.

Data-parallel over 8 NeuronCores: 64 samples each, shapelets replicated.
"""

import os
import sys

import numpy as np

for _p in ("/root/.axon_site/_ro/trn_rl_repo", "/opt/trn_rl_repo"):
    if os.path.isdir(_p) and _p not in sys.path:
        sys.path.append(_p)

B, C, T = 512, 1, 2048
S, L = 128, 64
NCORES = 8
BPC = B // NCORES          # samples per core = 64
P = T - L + 1              # 1985 windows
J = 32                     # j slots (j=31 is garbage except the edge)
CB, CBO = 16, 15           # even / odd transpose chunks

NMACC = 2                  # fp16 fold accumulators
NBACC = 2                  # fp16 direct-PSUM accumulators ('b' path)
# 46 'a' (ACT cast + DVE fold) / 18 'b' (DVE TT off PSUM) r's; the 'b's
# sit early (a,a,b pattern) so the acc merges hide in late-sweep slack.
PATHS = ["b" if (i % 3 == 2 and i <= 53) else "a" for i in range(64)]

_STATE = {}

_FLAGS = {"mains": True, "drain": True,
          "cast_only": False,  # 'a' path: cast but skip the fold (timing)
          "pe_x2": False,      # double each main matmul (timing probe)
          "cast4": False,      # split casts into quarters (timing probe)
          "ct_sw": False,      # CT assembly on gpsimd (SWDGE) as in v1
          "wn_mixed": False,   # Wn loads on sync/scalar/sync as in v1
          "unchunked": False}  # single x16 DMA + 2 transposes as in v1


def _build(nc, reps=1):
    import concourse.tile as tile
    from concourse import mybir

    f32 = mybir.dt.float32
    f16 = mybir.dt.float16
    OP = mybir.AluOpType
    AF = mybir.ActivationFunctionType
    AX = mybir.AxisListType.X

    x_d = nc.dram_tensor("x_shard", [BPC, T], f32, kind="ExternalInput").ap()
    wn_d = nc.dram_tensor("wn", [128, 65, S], f16, kind="ExternalInput").ap()
    wsq_d = nc.dram_tensor("wsq", [128, 65], f16, kind="ExternalInput").ap()
    wsm_d = nc.dram_tensor("wsm", [128, 65], f16, kind="ExternalInput").ap()
    nm_d = nc.dram_tensor("nmask", [128, 1], f16, kind="ExternalInput").ap()
    s2_d = nc.dram_tensor("s2v", [S, 1], f32, kind="ExternalInput").ap()
    id_d = nc.dram_tensor("ident", [128, 128], f32, kind="ExternalInput").ap()
    out_d = nc.dram_tensor("out", [BPC, S], f32, kind="ExternalOutput").ap()

    with tile.TileContext(nc) as tc:
      for _it in range(reps):
        with tc.tile_pool(name=f"const{_it}", bufs=1) as constp, \
             tc.tile_pool(name=f"big{_it}", bufs=1) as bigp:

            # Queue plan: ACT's ring must stay clear for the sweep casts,
            # and a dep-stalled ring entry blocks everything behind it.
            #   sync ring:   wsq/wsm (tiny, needed ~5us), XBAR transposes,
            #                CT x-row copies, out store.
            #   gpsimd ring: x16 cast-load, Wn (needed from ~7us), consts.
            #   scalar ring: CT A-row copies only (ready last).
            wsq = constp.tile([128, 65], f16)
            nc.sync.dma_start(wsq[:], wsq_d[:])
            wsm = constp.tile([128, 65], f16)
            nc.sync.dma_start(wsm[:], wsm_d[:])

            # x cast-loaded to fp16 (gpsimd DMAs convert dtypes)
            x16 = bigp.tile([BPC, T], f16)
            nc.gpsimd.dma_start(x16[:, 0:1088], x_d[:, 0:1088])
            nc.gpsimd.dma_start(x16[:, 1088:2048], x_d[:, 1088:2048])

            Wn = bigp.tile([128, 65, S], f16)
            nc.gpsimd.dma_start(Wn[:, 0:22], wn_d[:, 0:22])
            nc.gpsimd.dma_start(Wn[:, 22:44], wn_d[:, 22:44])
            nc.gpsimd.dma_start(Wn[:, 44:65], wn_d[:, 44:65])
            ident = constp.tile([128, 128], f32)
            nc.gpsimd.dma_start(ident[:], id_d[:])
            s2 = constp.tile([S, 1], f32)
            nc.gpsimd.dma_start(s2[:], s2_d[:])
            nmask = constp.tile([128, 1], f16)
            nc.gpsimd.dma_start(nmask[:], nm_d[:])

            xTe = bigp.tile([128, CB, BPC], f16)
            xTo = bigp.tile([128, CBO, BPC], f16)
            sqe = bigp.tile([128, CB, BPC], f16)
            sqo = bigp.tile([128, CBO, BPC], f16)
            if _FLAGS["unchunked"]:
                nc.sync.dma_start_transpose(xTe[:], x16[:, 0:2048])
                nc.sync.dma_start_transpose(xTo[:], x16[:, 64:1984])
                nc.vector.tensor_tensor(sqe[:], xTe[:], xTe[:], OP.mult)
                nc.vector.tensor_tensor(sqo[:], xTo[:], xTo[:], OP.mult)
            else:
                # chunked: the A-term pipeline starts after half the x load
                nc.sync.dma_start_transpose(xTe[:, 0:8], x16[:, 0:1024])
                nc.sync.dma_start_transpose(xTo[:, 0:8], x16[:, 64:1088])
                nc.sync.dma_start_transpose(xTe[:, 8:16], x16[:, 1024:2048])
                nc.sync.dma_start_transpose(xTo[:, 8:15], x16[:, 1088:1984])
                # squares on DVE (2x_1P) -- ACT is the drain bottleneck
                nc.vector.tensor_tensor(sqe[:, 0:8], xTe[:, 0:8],
                                        xTe[:, 0:8], OP.mult)
                nc.vector.tensor_tensor(sqo[:, 0:8], xTo[:, 0:8],
                                        xTo[:, 0:8], OP.mult)
                nc.vector.tensor_tensor(sqe[:, 8:16], xTe[:, 8:16],
                                        xTe[:, 8:16], OP.mult)
                nc.vector.tensor_tensor(sqo[:, 8:15], xTo[:, 8:15],
                                        xTo[:, 8:15], OP.mult)

            # fp16 fold accumulators (init by copy from the first casts);
            # 31 j-slots — the garbage column is never drained
            n_a = (PATHS.count("a")
                   if _FLAGS["drain"] and not _FLAGS["cast_only"] else 0)
            maccs = [bigp.tile([S, J - 1, BPC], f16, name=f"macc{i}")
                     for i in range(min(NMACC, n_a))]
            n_b = PATHS.count("b") if _FLAGS["drain"] else 0
            baccs = [bigp.tile([S, J - 1, BPC], f16, name=f"bacc{i}")
                     for i in range(min(NBACC, n_b))]
            binit = [[False, False] for _ in baccs]   # per (bacc, h) init
            # early-merge plan: valid for the default a,a,b pattern
            can_early = (len(maccs) == 2 and len(baccs) == 2
                         and n_a >= 4 and "b" not in PATHS[55:]
                         and PATHS[62] == "a" and PATHS[63] == "a")
            STR = bigp.tile([S, BPC, 1], f32)   # edge strip

            # CT layout [k, parity, c, b]: j = 2c + parity; (1,15) is garbage
            CT1 = bigp.tile([128, 2, CB, BPC], f16)
            CT2 = bigp.tile([128, 2, CB, BPC], f16)

            # ---- A = sum w^2 - (sum w)^2/64 via ones-weight matmuls,
            # chunked 0:8 / 8:16 to follow the transpose pipeline
            with tc.tile_pool(name=f"psA{_it}", bufs=1, space="PSUM") as psA:
                eSq = psA.tile([65, CB, BPC], f32)
                eSm = psA.tile([65, CB, BPC], f32)
                oSq = psA.tile([64, CBO, BPC], f32)
                oSm = psA.tile([64, CBO, BPC], f32)
                swE = constp.tile([65, CB, BPC], f32)
                swO = constp.tile([64, CBO, BPC], f32)
                ATe = bigp.tile([65, CB, BPC], f16)
                ATo = bigp.tile([64, CBO, BPC], f16)
                for c0, c1, c1o in ((0, 8, 8), (8, 16, 15)):
                    nc.tensor.matmul(eSq[:, c0:c1], wsq[:], sqe[:, c0:c1],
                                     start=True, stop=True)
                    nc.tensor.matmul(eSm[:, c0:c1], wsm[:], xTe[:, c0:c1],
                                     start=True, stop=True)
                    nc.tensor.matmul(oSq[:, c0:c1o], wsq[:, 0:64],
                                     sqo[:, c0:c1o], start=True, stop=True)
                    nc.tensor.matmul(oSm[:, c0:c1o], wsm[:, 0:64],
                                     xTo[:, c0:c1o], start=True, stop=True)
                    nc.scalar.activation(swE[:, c0:c1], eSm[:, c0:c1],
                                         AF.Square, scale=0.125)
                    nc.scalar.activation(swO[:, c0:c1o], oSm[:, c0:c1o],
                                         AF.Square, scale=0.125)
                    nc.vector.tensor_sub(ATe[:, c0:c1], eSq[:, c0:c1],
                                         swE[:, c0:c1])
                    nc.vector.tensor_sub(ATo[:, c0:c1o], oSq[:, c0:c1o],
                                         swO[:, c0:c1o])

            # ---- combined-tile assembly (HWDGE sync/scalar, or SWDGE),
            # chunked by transpose half so CT1's first columns are ready
            # as soon as the first x chunks land.
            ctq1 = nc.gpsimd if _FLAGS["ct_sw"] else nc.sync
            ctq2 = nc.gpsimd if _FLAGS["ct_sw"] else nc.scalar
            # x rows (even c 0:8 from chunk 1 first, then the rest)
            ctq1.dma_start(CT1[0:96, 0, 0:8], xTe[0:96, 0:8])
            ctq1.dma_start(CT1[0:96, 1, 0:8], xTo[0:96, 0:8])
            ctq1.dma_start(CT1[0:96, 0, 8:16], xTe[0:96, 8:16])
            ctq1.dma_start(CT1[0:96, 1, 8:15], xTo[0:96, 8:15])
            ctq1.dma_start(CT2[32:128, 0], xTe[32:128])
            ctq1.dma_start(CT2[32:128, 1, 0:15], xTo[32:128])
            # A rows (chunked likewise; CT2's A rows are needed ~mid-sweep)
            ctq2.dma_start(CT1[96:128, 0, 0:8], ATe[0:32, 0:8])
            ctq2.dma_start(CT1[96:128, 1, 0:8], ATo[0:32, 0:8])
            ctq2.dma_start(CT1[96:128, 0, 8:16], ATe[0:32, 8:16])
            ctq2.dma_start(CT1[96:128, 1, 8:15], ATo[0:32, 8:15])
            ctq2.dma_start(CT2[0:32, 0], ATe[32:64])
            ctq2.dma_start(CT2[0:32, 1, 0:15], ATo[32:64])
            # (the (1,15) garbage slot is never read: the odd-half second
            # matmul is N=448, so no memsets are needed)

            # ---- main sweep.  Default: per r, 4 matmuls into two 2-bank
            # PSUM tiles; 'a': two ACT casts + one DVE TT fold; 'b': two
            # DVE TT-max straight off PSUM into a (j,b) accumulator.
            with tc.tile_pool(name=f"psM{_it}", bufs=4, space="PSUM") as psM, \
                 tc.tile_pool(name=f"drain{_it}", bufs=4) as drp:
                # edge window p=1984 first: rows 64..127 of the j=30
                # column; its drain is one cheap DVE copy.
                pe = psM.tile([S, CB, BPC], f32, tag="ph")
                nc.tensor.matmul(pe[:, 0, :], Wn[:, 64, :],
                                 CT2[:, 0, 15], start=True, stop=False)
                # A[b,1984] sits at ATe[64, 15, b]; nmask is -1 at row 64
                nc.tensor.matmul(pe[:, 0, :],
                                 nmask[0:65].broadcast_to([65, S]),
                                 ATe[:, 15], start=False, stop=True)
                nc.vector.tensor_copy(STR[:, :, 0], pe[:, 0, :])

                na = 0
                nb = 0
                for r in range(64):
                    W = Wn[:, r, :]
                    CT = CT1 if r < 32 else CT2
                    path = PATHS[r] if _FLAGS["drain"] else "n"
                    sbm = (drp.tile([S, J - 1, BPC], f16, name="sbm",
                                    tag="sm") if path == "a" else None)
                    for h in range(2):
                        ph = psM.tile([S, CB, BPC], f32, tag="ph")
                        j0 = 16 * h
                        nv = 16 if h == 0 else 15   # drop the j=31 garbage
                        if _FLAGS["mains"]:
                            nc.tensor.matmul(ph[:, 0:8], W, CT[:, h, 0:8],
                                             start=True, stop=True)
                            nc.tensor.matmul(ph[:, 8:nv], W, CT[:, h, 8:nv],
                                             start=True, stop=True)
                            if _FLAGS["pe_x2"]:
                                nc.tensor.matmul(ph[:, 0:8], W,
                                                 CT[:, h, 0:8],
                                                 start=True, stop=True)
                                nc.tensor.matmul(ph[:, 8:nv], W,
                                                 CT[:, h, 8:nv],
                                                 start=True, stop=True)
                        if path == "a":
                            if _FLAGS["cast4"]:
                                nc.scalar.mul(sbm[:, j0:j0 + 8],
                                              ph[:, 0:8], 1.0)
                                nc.scalar.mul(sbm[:, j0 + 8:j0 + nv],
                                              ph[:, 8:nv], 1.0)
                            else:
                                nc.scalar.mul(sbm[:, j0:j0 + nv],
                                              ph[:, 0:nv], 1.0)
                        elif path == "b":
                            bi = nb % len(baccs)
                            bt = baccs[bi]
                            if not binit[bi][h]:
                                # first touch initialises by PSUM copy
                                nc.vector.tensor_copy(bt[:, j0:j0 + nv],
                                                      ph[:, 0:nv])
                                binit[bi][h] = True
                            else:
                                nc.vector.tensor_tensor(
                                    bt[:, j0:j0 + nv], ph[:, 0:nv],
                                    bt[:, j0:j0 + nv], OP.max)
                    if path == "a":
                        if maccs:
                            idx = na % len(maccs)
                            if can_early and na >= n_a - 2:
                                idx = (na + 1) % 2
                            m16 = maccs[idx]
                            if na < len(maccs):
                                nc.vector.tensor_copy(m16[:], sbm[:])
                            else:
                                nc.vector.tensor_tensor(m16[:], sbm[:],
                                                        m16[:], OP.max)
                            if can_early and na == n_a - 2:
                                # macc1 is final; merge into macc0 while
                                # the last cast runs
                                nc.vector.tensor_tensor(
                                    maccs[0][:], maccs[0][:], maccs[1][:],
                                    OP.max)
                        na += 1
                    elif path == "b":
                        nb += 1
                    if can_early and r == 54:
                        # all 'b' r's done by r=53: merge baccs in slack
                        nc.vector.tensor_tensor(baccs[0][:], baccs[0][:],
                                                baccs[1][:], OP.max)
                    if can_early and r == 56 and maccs:
                        nc.vector.tensor_tensor(maccs[0][:], maccs[0][:],
                                                baccs[0][:], OP.max)

                # ---- finish
                r16 = constp.tile([S, BPC], f32)
                if can_early and maccs:
                    folds = [maccs[0]]
                else:
                    folds = list(maccs) + list(baccs)
                if folds:
                    for t in folds[1:]:
                        nc.vector.tensor_tensor(folds[0][:], folds[0][:],
                                                t[:], OP.max)
                    nc.vector.tensor_reduce(
                        r16[:], folds[0][:].rearrange("p j b -> p b j"),
                        axis=AX, op=OP.max)
                    nc.vector.tensor_tensor(r16[:], r16[:], STR[:, :, 0],
                                            OP.max)
                else:
                    nc.vector.tensor_copy(r16[:], STR[:, :, 0])
                # d = sqrt(relu(s2 - v)):  (v - s2) clamped <= 0, Sqrt(-x)
                nc.vector.tensor_scalar(r16[:], r16[:], s2[:], 0.0,
                                        OP.subtract, OP.min)
                res = constp.tile([S, BPC], f32)
                nc.scalar.activation(res[:], r16[:], AF.Sqrt, scale=-1.0)

            with tc.tile_pool(name=f"psC{_it}", bufs=1, space="PSUM") as psC:
                po = psC.tile([BPC, S], f32)
                nc.tensor.transpose(po[:], res[:], ident[:])
                outsb = constp.tile([BPC, S], f32)
                nc.scalar.mul(outsb[:], po[:], 1.0)
                nc.sync.dma_start(out_d[:], outsb[:])


def _wn_np(sh):
    # sh: (S, L) float32 -> Wn (128, 65, S) fp16:
    #   rows [r, r+64) of slot r hold +2 s~[s, k-r]; indicator row -1.
    st = 2.0 * (sh - sh.mean(axis=1, keepdims=True))       # (S, L)
    wn = np.zeros((128, 65, S), dtype=np.float32)
    for r in range(65):
        wn[r:r + 64, r, :] = st.T
        if r < 32:
            wn[96 + r, r, :] = -1.0
        elif r < 64:
            wn[r - 32, r, :] = -1.0
    return wn.astype(np.float16)


def _wins_np():
    # ones-window weights: wsq[k, m] = 1 for k in [m, m+64)
    w = np.zeros((128, 65), dtype=np.float16)
    for m in range(65):
        w[m:m + 64, m] = 1.0
    return w


def _get_nc():
    if "nc" not in _STATE:
        from concourse import bacc
        nc = bacc.Bacc("TRN2", target_bir_lowering=False, debug=False,
                       num_devices=NCORES)
        _build(nc)
        nc.compile()
        _STATE["nc"] = nc
    return _STATE["nc"]


def _in_maps(x, shapelets):
    x = np.ascontiguousarray(np.asarray(x, dtype=np.float32)).reshape(B, T)
    sh = np.ascontiguousarray(
        np.asarray(shapelets, dtype=np.float32)).reshape(S, L)
    wn = _wn_np(sh)
    wsq = _wins_np()
    nmask = np.zeros((128, 1), dtype=np.float16)
    nmask[64, 0] = -1.0
    s2v = (sh * sh).sum(axis=1, dtype=np.float32).reshape(S, 1)
    ident = np.eye(128, dtype=np.float32)
    return [{"x_shard": x[i * BPC:(i + 1) * BPC], "wn": wn, "wsq": wsq,
             "wsm": wsq, "nmask": nmask, "s2v": s2v, "ident": ident}
            for i in range(NCORES)]


def kernel(x, shapelets):
    from concourse.bass_utils import run_bass_kernel_spmd
    nc = _get_nc()
    res = run_bass_kernel_spmd(nc, _in_maps(x, shapelets),
                               list(range(NCORES))).results
    return np.concatenate([res[i]["out"] for i in range(NCORES)], axis=0)


if __name__ == "__main__":
    rng = np.random.default_rng(0)
    x = rng.standard_normal((B, C, T)).astype(np.float32)
    sh = rng.standard_normal((S, C, L)).astype(np.float32)
    out = kernel(x, sh)
    print("out", out.shape, out.dtype, float(out.min()), float(out.max()))


# revision 9
# speedup vs baseline: 2.6324x; 2.6324x over previous
"""Trainium2 Bass kernel for nn_DistanceLayer (shapelet min-distance), v2.

reference semantics:
  x: (512, 1, 2048), shapelets: (128, 1, 64)
  patches = sliding windows of x (len 64, stride 1), mean-centered
  out[b, s] = min_p ||patch(b, p) - shapelets[s]||_2          -> (512, 128)

Math (negated domain so the reduction is a MAX):
  With s~ = sh - mean_l(sh):  (w - mean(w)) . sh = w . s~
    d2[b,s,p] = A[b,p] + s2[s] - 2 w.s~,  A = sum(w^2) - (sum w)^2/64
  PE computes  v = 2 w.s~ - A  per (s, window); max_p v = s2 - min_p d2,
  so  out = sqrt(relu(s2 - max_p v)).

v2 drain strategy (HW-measured op costs): PSUM can only exit via ACT
(1 elem/lane/cyc @1.2GHz) or DVE (@0.96GHz), so the 16.3M-value v
matrix is split between two drain paths balanced by an LP:
  'a' (46 r's): two ACT casts PSUM fp32 -> SBUF fp16 (~1us each,
      (172+FD) cyc), then one DVE tensor_tensor max fold (2x_1P on
      fp16 SBUF, ~1.1us per r) into one of two alternating fp16
      accumulators;
  'b' (18 r's): two DVE tensor_tensor max ops straight off PSUM into
      a (j,b)-layout fp16 accumulator (~1.2us each, no ACT).
The v1 'd' path (strided tensor_reduce from PSUM, ~2.9us/r in situ)
and STT folds (TensorScalarPtr is 1x on HW, not the hoped 4x; plain
TT gets 2x_1P) are gone.  ACT ~95us and DVE ~100us busy bracket the
span; PE (~60us) hides underneath.  All accumulator merges are
emitted mid-sweep (r=54/56/62) so the tail after the last cast is
just one reduce + sqrt + PE transpose + store.

Setup: ACT's queue stays clear of DMAs (a dep-stalled ring entry
blocks everything behind it): sync ring carries the XBAR transposes +
CT x-row copies, gpsimd (SWDGE) carries the x16 cast-load + Wn +
consts, scalar carries only the late CT A-row copies.  x16 load,
transposes, squares (on DVE) and the A-term pipeline are chunked
two ways so first mains start after half the x load; the j=31
garbage column was eliminated (odd-half second matmul is N=448), so
no garbage memsets exist.

Data-parallel over 8 NeuronCores: 64 samples each, shapelets replicated.
"""

import os
import sys

import numpy as np

for _p in ("/root/.axon_site/_ro/trn_rl_repo", "/opt/trn_rl_repo"):
    if os.path.isdir(_p) and _p not in sys.path:
        sys.path.append(_p)

B, C, T = 512, 1, 2048
S, L = 128, 64
NCORES = 8
BPC = B // NCORES          # samples per core = 64
P = T - L + 1              # 1985 windows
J = 32                     # j slots (j=31 is garbage except the edge)
CB, CBO = 16, 15           # even / odd transpose chunks

NMACC = 2                  # fp16 fold accumulators
NBACC = 2                  # fp16 direct-PSUM accumulators ('b' path)
# 46 'a' (ACT cast + DVE fold) / 18 'b' (DVE TT off PSUM) r's; the 'b's
# sit early (a,a,b pattern) so the acc merges hide in late-sweep slack.
PATHS = ["b" if (i % 3 == 2 and i <= 53) else "a" for i in range(64)]

_STATE = {}

_FLAGS = {"mains": True, "drain": True,
          "cast_only": False,  # 'a' path: cast but skip the fold (timing)
          "pe_x2": False,      # double each main matmul (timing probe)
          "cast4": False,      # split casts into quarters (timing probe)
          "ct_sw": False,      # CT assembly on gpsimd (SWDGE) as in v1
          "wn_mixed": False,   # Wn loads on sync/scalar/sync as in v1
          "unchunked": False}  # single x16 DMA + 2 transposes as in v1


def _build(nc, reps=1):
    import concourse.tile as tile
    from concourse import mybir

    f32 = mybir.dt.float32
    f16 = mybir.dt.float16
    OP = mybir.AluOpType
    AF = mybir.ActivationFunctionType
    AX = mybir.AxisListType.X

    x_d = nc.dram_tensor("x_shard", [BPC, T], f32, kind="ExternalInput").ap()
    wn_d = nc.dram_tensor("wn", [128, 65, S], f16, kind="ExternalInput").ap()
    wsq_d = nc.dram_tensor("wsq", [128, 65], f16, kind="ExternalInput").ap()
    wsm_d = nc.dram_tensor("wsm", [128, 65], f16, kind="ExternalInput").ap()
    nm_d = nc.dram_tensor("nmask", [128, 1], f16, kind="ExternalInput").ap()
    s2_d = nc.dram_tensor("s2v", [S, 1], f32, kind="ExternalInput").ap()
    id_d = nc.dram_tensor("ident", [128, 128], f32, kind="ExternalInput").ap()
    out_d = nc.dram_tensor("out", [BPC, S], f32, kind="ExternalOutput").ap()

    with tile.TileContext(nc) as tc:
      for _it in range(reps):
        with tc.tile_pool(name=f"const{_it}", bufs=1) as constp, \
             tc.tile_pool(name=f"big{_it}", bufs=1) as bigp:

            # Queue plan: ACT's ring must stay clear for the sweep casts,
            # and a dep-stalled ring entry blocks everything behind it.
            #   sync ring:   wsq/wsm (tiny, needed ~5us), XBAR transposes,
            #                CT x-row copies, out store.
            #   gpsimd ring: x16 cast-load, Wn (needed from ~7us), consts.
            #   scalar ring: CT A-row copies only (ready last).
            wsq = constp.tile([128, 65], f16)
            nc.sync.dma_start(wsq[:], wsq_d[:])
            wsm = constp.tile([128, 65], f16)
            nc.sync.dma_start(wsm[:], wsm_d[:])

            # x cast-loaded to fp16 (gpsimd DMAs convert dtypes)
            x16 = bigp.tile([BPC, T], f16)
            nc.gpsimd.dma_start(x16[:, 0:1088], x_d[:, 0:1088])
            nc.gpsimd.dma_start(x16[:, 1088:2048], x_d[:, 1088:2048])

            # Wn on the scalar (HWDGE) ring: keeping these big transfers
            # off the SWDGE queue lets the CT x-row copies start ~5us
            # earlier (they serialize against outstanding SWDGE work via
            # the transpose guard); the CT A-row copies queued behind Wn
            # on this ring are dep-stalled until later anyway.
            Wn = bigp.tile([128, 65, S], f16)
            nc.scalar.dma_start(Wn[:, 0:22], wn_d[:, 0:22])
            nc.scalar.dma_start(Wn[:, 22:44], wn_d[:, 22:44])
            nc.scalar.dma_start(Wn[:, 44:65], wn_d[:, 44:65])
            ident = constp.tile([128, 128], f32)
            nc.gpsimd.dma_start(ident[:], id_d[:])
            s2 = constp.tile([S, 1], f32)
            nc.gpsimd.dma_start(s2[:], s2_d[:])
            nmask = constp.tile([128, 1], f16)
            nc.gpsimd.dma_start(nmask[:], nm_d[:])

            xTe = bigp.tile([128, CB, BPC], f16)
            xTo = bigp.tile([128, CBO, BPC], f16)
            sqe = bigp.tile([128, CB, BPC], f16)
            sqo = bigp.tile([128, CBO, BPC], f16)
            if _FLAGS["unchunked"]:
                nc.sync.dma_start_transpose(xTe[:], x16[:, 0:2048])
                nc.sync.dma_start_transpose(xTo[:], x16[:, 64:1984])
                nc.vector.tensor_tensor(sqe[:], xTe[:], xTe[:], OP.mult)
                nc.vector.tensor_tensor(sqo[:], xTo[:], xTo[:], OP.mult)
            else:
                # chunked: the A-term pipeline starts after half the x load
                nc.sync.dma_start_transpose(xTe[:, 0:8], x16[:, 0:1024])
                nc.sync.dma_start_transpose(xTo[:, 0:8], x16[:, 64:1088])
                nc.sync.dma_start_transpose(xTe[:, 8:16], x16[:, 1024:2048])
                nc.sync.dma_start_transpose(xTo[:, 8:15], x16[:, 1088:1984])
                # squares on DVE (2x_1P) -- ACT is the drain bottleneck
                nc.vector.tensor_tensor(sqe[:, 0:8], xTe[:, 0:8],
                                        xTe[:, 0:8], OP.mult)
                nc.vector.tensor_tensor(sqo[:, 0:8], xTo[:, 0:8],
                                        xTo[:, 0:8], OP.mult)
                nc.vector.tensor_tensor(sqe[:, 8:16], xTe[:, 8:16],
                                        xTe[:, 8:16], OP.mult)
                nc.vector.tensor_tensor(sqo[:, 8:15], xTo[:, 8:15],
                                        xTo[:, 8:15], OP.mult)

            # fp16 fold accumulators (init by copy from the first casts);
            # 31 j-slots — the garbage column is never drained
            n_a = (PATHS.count("a")
                   if _FLAGS["drain"] and not _FLAGS["cast_only"] else 0)
            maccs = [bigp.tile([S, J - 1, BPC], f16, name=f"macc{i}")
                     for i in range(min(NMACC, n_a))]
            n_b = PATHS.count("b") if _FLAGS["drain"] else 0
            baccs = [bigp.tile([S, J - 1, BPC], f16, name=f"bacc{i}")
                     for i in range(min(NBACC, n_b))]
            binit = [[False, False] for _ in baccs]   # per (bacc, h) init
            # early-merge plan: valid for the default a,a,b pattern
            can_early = (len(maccs) == 2 and len(baccs) == 2
                         and n_a >= 4 and "b" not in PATHS[55:]
                         and PATHS[62] == "a" and PATHS[63] == "a")
            STR = bigp.tile([S, BPC, 1], f32)   # edge strip

            # CT layout [k, parity, c, b]: j = 2c + parity; (1,15) is garbage
            CT1 = bigp.tile([128, 2, CB, BPC], f16)
            CT2 = bigp.tile([128, 2, CB, BPC], f16)

            # ---- A = sum w^2 - (sum w)^2/64 via ones-weight matmuls,
            # chunked 0:8 / 8:16 to follow the transpose pipeline
            with tc.tile_pool(name=f"psA{_it}", bufs=1, space="PSUM") as psA:
                eSq = psA.tile([65, CB, BPC], f32)
                eSm = psA.tile([65, CB, BPC], f32)
                oSq = psA.tile([64, CBO, BPC], f32)
                oSm = psA.tile([64, CBO, BPC], f32)
                swE = constp.tile([65, CB, BPC], f32)
                swO = constp.tile([64, CBO, BPC], f32)
                ATe = bigp.tile([65, CB, BPC], f16)
                ATo = bigp.tile([64, CBO, BPC], f16)
                for c0, c1, c1o in ((0, 8, 8), (8, 16, 15)):
                    nc.tensor.matmul(eSq[:, c0:c1], wsq[:], sqe[:, c0:c1],
                                     start=True, stop=True)
                    nc.tensor.matmul(eSm[:, c0:c1], wsm[:], xTe[:, c0:c1],
                                     start=True, stop=True)
                    nc.tensor.matmul(oSq[:, c0:c1o], wsq[:, 0:64],
                                     sqo[:, c0:c1o], start=True, stop=True)
                    nc.tensor.matmul(oSm[:, c0:c1o], wsm[:, 0:64],
                                     xTo[:, c0:c1o], start=True, stop=True)
                    nc.scalar.activation(swE[:, c0:c1], eSm[:, c0:c1],
                                         AF.Square, scale=0.125)
                    nc.scalar.activation(swO[:, c0:c1o], oSm[:, c0:c1o],
                                         AF.Square, scale=0.125)
                    nc.vector.tensor_sub(ATe[:, c0:c1], eSq[:, c0:c1],
                                         swE[:, c0:c1])
                    nc.vector.tensor_sub(ATo[:, c0:c1o], oSq[:, c0:c1o],
                                         swO[:, c0:c1o])

            # ---- combined-tile assembly (HWDGE sync/scalar, or SWDGE),
            # chunked by transpose half so CT1's first columns are ready
            # as soon as the first x chunks land.
            ctq1 = nc.gpsimd if _FLAGS["ct_sw"] else nc.sync
            ctq2 = nc.gpsimd if _FLAGS["ct_sw"] else nc.scalar
            # x rows (even c 0:8 from chunk 1 first, then the rest)
            ctq1.dma_start(CT1[0:96, 0, 0:8], xTe[0:96, 0:8])
            ctq1.dma_start(CT1[0:96, 1, 0:8], xTo[0:96, 0:8])
            ctq1.dma_start(CT1[0:96, 0, 8:16], xTe[0:96, 8:16])
            ctq1.dma_start(CT1[0:96, 1, 8:15], xTo[0:96, 8:15])
            ctq1.dma_start(CT2[32:128, 0], xTe[32:128])
            ctq1.dma_start(CT2[32:128, 1, 0:15], xTo[32:128])
            # A rows (chunked likewise; CT2's A rows are needed ~mid-sweep)
            ctq2.dma_start(CT1[96:128, 0, 0:8], ATe[0:32, 0:8])
            ctq2.dma_start(CT1[96:128, 1, 0:8], ATo[0:32, 0:8])
            ctq2.dma_start(CT1[96:128, 0, 8:16], ATe[0:32, 8:16])
            ctq2.dma_start(CT1[96:128, 1, 8:15], ATo[0:32, 8:15])
            ctq2.dma_start(CT2[0:32, 0], ATe[32:64])
            ctq2.dma_start(CT2[0:32, 1, 0:15], ATo[32:64])
            # (the (1,15) garbage slot is never read: the odd-half second
            # matmul is N=448, so no memsets are needed)

            # ---- main sweep.  Default: per r, 4 matmuls into two 2-bank
            # PSUM tiles; 'a': two ACT casts + one DVE TT fold; 'b': two
            # DVE TT-max straight off PSUM into a (j,b) accumulator.
            with tc.tile_pool(name=f"psM{_it}", bufs=4, space="PSUM") as psM, \
                 tc.tile_pool(name=f"drain{_it}", bufs=4) as drp:
                # edge window p=1984 first: rows 64..127 of the j=30
                # column; its drain is one cheap DVE copy.
                pe = psM.tile([S, CB, BPC], f32, tag="ph")
                nc.tensor.matmul(pe[:, 0, :], Wn[:, 64, :],
                                 CT2[:, 0, 15], start=True, stop=False)
                # A[b,1984] sits at ATe[64, 15, b]; nmask is -1 at row 64
                nc.tensor.matmul(pe[:, 0, :],
                                 nmask[0:65].broadcast_to([65, S]),
                                 ATe[:, 15], start=False, stop=True)
                nc.vector.tensor_copy(STR[:, :, 0], pe[:, 0, :])

                na = 0
                nb = 0
                for r in range(64):
                    W = Wn[:, r, :]
                    CT = CT1 if r < 32 else CT2
                    path = PATHS[r] if _FLAGS["drain"] else "n"
                    sbm = (drp.tile([S, J - 1, BPC], f16, name="sbm",
                                    tag="sm") if path == "a" else None)
                    for h in range(2):
                        ph = psM.tile([S, CB, BPC], f32, tag="ph")
                        j0 = 16 * h
                        nv = 16 if h == 0 else 15   # drop the j=31 garbage
                        if _FLAGS["mains"]:
                            nc.tensor.matmul(ph[:, 0:8], W, CT[:, h, 0:8],
                                             start=True, stop=True)
                            nc.tensor.matmul(ph[:, 8:nv], W, CT[:, h, 8:nv],
                                             start=True, stop=True)
                            if _FLAGS["pe_x2"]:
                                nc.tensor.matmul(ph[:, 0:8], W,
                                                 CT[:, h, 0:8],
                                                 start=True, stop=True)
                                nc.tensor.matmul(ph[:, 8:nv], W,
                                                 CT[:, h, 8:nv],
                                                 start=True, stop=True)
                        if path == "a":
                            if _FLAGS["cast4"]:
                                nc.scalar.mul(sbm[:, j0:j0 + 8],
                                              ph[:, 0:8], 1.0)
                                nc.scalar.mul(sbm[:, j0 + 8:j0 + nv],
                                              ph[:, 8:nv], 1.0)
                            else:
                                nc.scalar.mul(sbm[:, j0:j0 + nv],
                                              ph[:, 0:nv], 1.0)
                        elif path == "b":
                            bi = nb % len(baccs)
                            bt = baccs[bi]
                            if not binit[bi][h]:
                                # first touch initialises by PSUM copy
                                nc.vector.tensor_copy(bt[:, j0:j0 + nv],
                                                      ph[:, 0:nv])
                                binit[bi][h] = True
                            else:
                                nc.vector.tensor_tensor(
                                    bt[:, j0:j0 + nv], ph[:, 0:nv],
                                    bt[:, j0:j0 + nv], OP.max)
                    if path == "a":
                        if maccs:
                            idx = na % len(maccs)
                            if can_early and na >= n_a - 2:
                                idx = (na + 1) % 2
                            m16 = maccs[idx]
                            if na < len(maccs):
                                nc.vector.tensor_copy(m16[:], sbm[:])
                            else:
                                nc.vector.tensor_tensor(m16[:], sbm[:],
                                                        m16[:], OP.max)
                            if can_early and na == n_a - 2:
                                # macc1 is final; merge into macc0 while
                                # the last cast runs
                                nc.vector.tensor_tensor(
                                    maccs[0][:], maccs[0][:], maccs[1][:],
                                    OP.max)
                        na += 1
                    elif path == "b":
                        nb += 1
                    if can_early and r == 54:
                        # all 'b' r's done by r=53: merge baccs in slack
                        nc.vector.tensor_tensor(baccs[0][:], baccs[0][:],
                                                baccs[1][:], OP.max)
                    if can_early and r == 56 and maccs:
                        nc.vector.tensor_tensor(maccs[0][:], maccs[0][:],
                                                baccs[0][:], OP.max)

                # ---- finish
                r16 = constp.tile([S, BPC], f32)
                if can_early and maccs:
                    folds = [maccs[0]]
                else:
                    folds = list(maccs) + list(baccs)
                if folds:
                    for t in folds[1:]:
                        nc.vector.tensor_tensor(folds[0][:], folds[0][:],
                                                t[:], OP.max)
                    nc.vector.tensor_reduce(
                        r16[:], folds[0][:].rearrange("p j b -> p b j"),
                        axis=AX, op=OP.max)
                    nc.vector.tensor_tensor(r16[:], r16[:], STR[:, :, 0],
                                            OP.max)
                else:
                    nc.vector.tensor_copy(r16[:], STR[:, :, 0])
                # d = sqrt(relu(s2 - v)):  (v - s2) clamped <= 0, Sqrt(-x)
                nc.vector.tensor_scalar(r16[:], r16[:], s2[:], 0.0,
                                        OP.subtract, OP.min)
                res = constp.tile([S, BPC], f32)
                nc.scalar.activation(res[:], r16[:], AF.Sqrt, scale=-1.0)

            with tc.tile_pool(name=f"psC{_it}", bufs=1, space="PSUM") as psC:
                po = psC.tile([BPC, S], f32)
                nc.tensor.transpose(po[:], res[:], ident[:])
                outsb = constp.tile([BPC, S], f32)
                nc.scalar.mul(outsb[:], po[:], 1.0)
                nc.sync.dma_start(out_d[:], outsb[:])


def _wn_np(sh):
    # sh: (S, L) float32 -> Wn (128, 65, S) fp16:
    #   rows [r, r+64) of slot r hold +2 s~[s, k-r]; indicator row -1.
    st = 2.0 * (sh - sh.mean(axis=1, keepdims=True))       # (S, L)
    wn = np.zeros((128, 65, S), dtype=np.float32)
    for r in range(65):
        wn[r:r + 64, r, :] = st.T
        if r < 32:
            wn[96 + r, r, :] = -1.0
        elif r < 64:
            wn[r - 32, r, :] = -1.0
    return wn.astype(np.float16)


def _wins_np():
    # ones-window weights: wsq[k, m] = 1 for k in [m, m+64)
    w = np.zeros((128, 65), dtype=np.float16)
    for m in range(65):
        w[m:m + 64, m] = 1.0
    return w


def _get_nc():
    if "nc" not in _STATE:
        from concourse import bacc
        nc = bacc.Bacc("TRN2", target_bir_lowering=False, debug=False,
                       num_devices=NCORES)
        _build(nc)
        nc.compile()
        _STATE["nc"] = nc
    return _STATE["nc"]


def _in_maps(x, shapelets):
    x = np.ascontiguousarray(np.asarray(x, dtype=np.float32)).reshape(B, T)
    sh = np.ascontiguousarray(
        np.asarray(shapelets, dtype=np.float32)).reshape(S, L)
    wn = _wn_np(sh)
    wsq = _wins_np()
    nmask = np.zeros((128, 1), dtype=np.float16)
    nmask[64, 0] = -1.0
    s2v = (sh * sh).sum(axis=1, dtype=np.float32).reshape(S, 1)
    ident = np.eye(128, dtype=np.float32)
    return [{"x_shard": x[i * BPC:(i + 1) * BPC], "wn": wn, "wsq": wsq,
             "wsm": wsq, "nmask": nmask, "s2v": s2v, "ident": ident}
            for i in range(NCORES)]


def kernel(x, shapelets):
    from concourse.bass_utils import run_bass_kernel_spmd
    nc = _get_nc()
    res = run_bass_kernel_spmd(nc, _in_maps(x, shapelets),
                               list(range(NCORES))).results
    return np.concatenate([res[i]["out"] for i in range(NCORES)], axis=0)


if __name__ == "__main__":
    rng = np.random.default_rng(0)
    x = rng.standard_normal((B, C, T)).astype(np.float32)
    sh = rng.standard_normal((S, C, L)).astype(np.float32)
    out = kernel(x, sh)
    print("out", out.shape, out.dtype, float(out.min()), float(out.max()))


# revision 10
# speedup vs baseline: 5.1214x; 1.9455x over previous
"""Trainium2 Bass kernel for nn_DistanceLayer (shapelet min-distance), v2.

reference semantics:
  x: (512, 1, 2048), shapelets: (128, 1, 64)
  patches = sliding windows of x (len 64, stride 1), mean-centered
  out[b, s] = min_p ||patch(b, p) - shapelets[s]||_2          -> (512, 128)

Math (negated domain so the reduction is a MAX):
  With s~ = sh - mean_l(sh):  (w - mean(w)) . sh = w . s~
    d2[b,s,p] = A[b,p] + s2[s] - 2 w.s~,  A = sum(w^2) - (sum w)^2/64
  PE computes  v = 2 w.s~ - A  per (s, window); max_p v = s2 - min_p d2,
  so  out = sqrt(relu(s2 - max_p v)).

v2 drain strategy (HW-measured op costs): PSUM can only exit via ACT
(1 elem/lane/cyc @1.2GHz) or DVE (@0.96GHz), so the 16.3M-value v
matrix is split between two drain paths balanced by an LP:
  'a' (46 r's): two ACT casts PSUM fp32 -> SBUF fp16 (~1us each,
      (172+FD) cyc), then one DVE tensor_tensor max fold (2x_1P on
      fp16 SBUF, ~1.1us per r) into one of two alternating fp16
      accumulators;
  'b' (18 r's): two DVE tensor_tensor max ops straight off PSUM into
      a (j,b)-layout fp16 accumulator (~1.2us each, no ACT).
The v1 'd' path (strided tensor_reduce from PSUM, ~2.9us/r in situ)
and STT folds (TensorScalarPtr is 1x on HW, not the hoped 4x; plain
TT gets 2x_1P) are gone.  ACT ~95us and DVE ~100us busy bracket the
span; PE (~60us) hides underneath.  All accumulator merges are
emitted mid-sweep (r=54/56/62) so the tail after the last cast is
just one reduce + sqrt + PE transpose + store.

Setup: ACT's queue stays clear of DMAs (a dep-stalled ring entry
blocks everything behind it): sync ring carries the XBAR transposes +
CT x-row copies, gpsimd (SWDGE) carries the x16 cast-load + Wn +
consts, scalar carries only the late CT A-row copies.  x16 load,
transposes, squares (on DVE) and the A-term pipeline are chunked
two ways so first mains start after half the x load; the j=31
garbage column was eliminated (odd-half second matmul is N=448), so
no garbage memsets exist.

Data-parallel over 8 NeuronCores: 64 samples each, shapelets replicated.
"""

import os
import sys

import numpy as np

for _p in ("/root/.axon_site/_ro/trn_rl_repo", "/opt/trn_rl_repo"):
    if os.path.isdir(_p) and _p not in sys.path:
        sys.path.append(_p)

B, C, T = 512, 1, 2048
S, L = 128, 64
NCORES = 8
BPC = B // NCORES          # samples per core = 64
P = T - L + 1              # 1985 windows
J = 32                     # j slots (j=31 is garbage except the edge)
CB, CBO = 16, 15           # even / odd transpose chunks

NMACC = 2                  # fp16 fold accumulators
NBACC = 2                  # fp16 direct-PSUM accumulators ('b' path)
# 46 'a' (ACT cast + DVE fold) / 18 'b' (DVE TT off PSUM) r's; the 'b's
# sit early (a,a,b pattern) so the acc merges hide in late-sweep slack.
PATHS = ["b" if (i % 3 == 2 and i <= 53) else "a" for i in range(64)]

_STATE = {}

_FLAGS = {"mains": True, "drain": True,
          "cast_only": False,  # 'a' path: cast but skip the fold (timing)
          "pe_x2": False,      # double each main matmul (timing probe)
          "cast4": False,      # split casts into quarters (timing probe)
          "ct_sw": False,      # CT assembly on gpsimd (SWDGE) as in v1
          "wn_mixed": False,   # Wn loads on sync/scalar/sync as in v1
          "unchunked": False}  # single x16 DMA + 2 transposes as in v1


def _build(nc, reps=1):
    import concourse.tile as tile
    from concourse import mybir

    f32 = mybir.dt.float32
    f16 = mybir.dt.float16
    OP = mybir.AluOpType
    AF = mybir.ActivationFunctionType
    AX = mybir.AxisListType.X

    x_d = nc.dram_tensor("x_shard", [BPC, T], f32, kind="ExternalInput").ap()
    wn_d = nc.dram_tensor("wn", [128, 65, S], f16, kind="ExternalInput").ap()
    wsq_d = nc.dram_tensor("wsq", [128, 65], f16, kind="ExternalInput").ap()
    wsm_d = nc.dram_tensor("wsm", [128, 65], f16, kind="ExternalInput").ap()
    nm_d = nc.dram_tensor("nmask", [128, 1], f16, kind="ExternalInput").ap()
    s2_d = nc.dram_tensor("s2v", [S, 1], f32, kind="ExternalInput").ap()
    id_d = nc.dram_tensor("ident", [128, 128], f32, kind="ExternalInput").ap()
    out_d = nc.dram_tensor("out", [BPC, S], f32, kind="ExternalOutput").ap()

    with tile.TileContext(nc) as tc:
      for _it in range(reps):
        with tc.tile_pool(name=f"const{_it}", bufs=1) as constp, \
             tc.tile_pool(name=f"big{_it}", bufs=1) as bigp:

            # Queue plan: ACT's ring must stay clear for the sweep casts,
            # and a dep-stalled ring entry blocks everything behind it.
            #   sync ring:   wsq/wsm (tiny, needed ~5us), XBAR transposes,
            #                CT x-row copies, out store.
            #   gpsimd ring: x16 cast-load, Wn (needed from ~7us), consts.
            #   scalar ring: CT A-row copies only (ready last).
            wsq = constp.tile([128, 65], f16)
            nc.sync.dma_start(wsq[:], wsq_d[:])
            wsm = constp.tile([128, 65], f16)
            nc.sync.dma_start(wsm[:], wsm_d[:])

            # x cast-loaded to fp16 (gpsimd DMAs convert dtypes)
            x16 = bigp.tile([BPC, T], f16)
            nc.gpsimd.dma_start(x16[:, 0:1088], x_d[:, 0:1088])
            nc.gpsimd.dma_start(x16[:, 1088:2048], x_d[:, 1088:2048])

            # Wn on the scalar (HWDGE) ring: keeping these big transfers
            # off the SWDGE queue lets the CT x-row copies start ~5us
            # earlier (they serialize against outstanding SWDGE work via
            # the transpose guard); the CT A-row copies queued behind Wn
            # on this ring are dep-stalled until later anyway.
            Wn = bigp.tile([128, 65, S], f16)
            nc.scalar.dma_start(Wn[:, 0:22], wn_d[:, 0:22])
            nc.scalar.dma_start(Wn[:, 22:44], wn_d[:, 22:44])
            nc.scalar.dma_start(Wn[:, 44:65], wn_d[:, 44:65])
            ident = constp.tile([128, 128], f32)
            nc.gpsimd.dma_start(ident[:], id_d[:])
            s2 = constp.tile([S, 1], f32)
            nc.gpsimd.dma_start(s2[:], s2_d[:])
            nmask = constp.tile([128, 1], f16)
            nc.gpsimd.dma_start(nmask[:], nm_d[:])

            xTe = bigp.tile([128, CB, BPC], f16)
            xTo = bigp.tile([128, CBO, BPC], f16)
            sqe = bigp.tile([128, CB, BPC], f16)
            sqo = bigp.tile([128, CBO, BPC], f16)
            if _FLAGS["unchunked"]:
                nc.sync.dma_start_transpose(xTe[:], x16[:, 0:2048])
                nc.sync.dma_start_transpose(xTo[:], x16[:, 64:1984])
                nc.vector.tensor_tensor(sqe[:], xTe[:], xTe[:], OP.mult)
                nc.vector.tensor_tensor(sqo[:], xTo[:], xTo[:], OP.mult)
            else:
                # chunked: the A-term pipeline starts after half the x load
                nc.sync.dma_start_transpose(xTe[:, 0:8], x16[:, 0:1024])
                nc.sync.dma_start_transpose(xTo[:, 0:8], x16[:, 64:1088])
                nc.sync.dma_start_transpose(xTe[:, 8:16], x16[:, 1024:2048])
                nc.sync.dma_start_transpose(xTo[:, 8:15], x16[:, 1088:1984])
                # squares on DVE (2x_1P) -- ACT is the drain bottleneck
                nc.vector.tensor_tensor(sqe[:, 0:8], xTe[:, 0:8],
                                        xTe[:, 0:8], OP.mult)
                nc.vector.tensor_tensor(sqo[:, 0:8], xTo[:, 0:8],
                                        xTo[:, 0:8], OP.mult)
                nc.vector.tensor_tensor(sqe[:, 8:16], xTe[:, 8:16],
                                        xTe[:, 8:16], OP.mult)
                nc.vector.tensor_tensor(sqo[:, 8:15], xTo[:, 8:15],
                                        xTo[:, 8:15], OP.mult)

            # fp16 fold accumulators (init by copy from the first casts);
            # 31 j-slots — the garbage column is never drained
            n_a = (PATHS.count("a")
                   if _FLAGS["drain"] and not _FLAGS["cast_only"] else 0)
            maccs = [bigp.tile([S, J - 1, BPC], f16, name=f"macc{i}")
                     for i in range(min(NMACC, n_a))]
            n_b = PATHS.count("b") if _FLAGS["drain"] else 0
            baccs = [bigp.tile([S, J - 1, BPC], f16, name=f"bacc{i}")
                     for i in range(min(NBACC, n_b))]
            binit = [[False, False] for _ in baccs]   # per (bacc, h) init
            # early-merge plan: valid for the default a,a,b pattern
            can_early = (len(maccs) == 2 and len(baccs) == 2
                         and n_a >= 4 and "b" not in PATHS[55:]
                         and PATHS[62] == "a" and PATHS[63] == "a")
            STR = bigp.tile([S, BPC, 1], f32)   # edge strip

            # CT layout [k, parity, c, b]: j = 2c + parity; (1,15) is garbage
            CT1 = bigp.tile([128, 2, CB, BPC], f16)
            CT2 = bigp.tile([128, 2, CB, BPC], f16)

            # ---- A = sum w^2 - (sum w)^2/64 via ones-weight matmuls,
            # chunked 0:8 / 8:16 to follow the transpose pipeline
            with tc.tile_pool(name=f"psA{_it}", bufs=1, space="PSUM") as psA:
                eSq = psA.tile([65, CB, BPC], f32)
                eSm = psA.tile([65, CB, BPC], f32)
                oSq = psA.tile([64, CBO, BPC], f32)
                oSm = psA.tile([64, CBO, BPC], f32)
                swE = constp.tile([65, CB, BPC], f32)
                swO = constp.tile([64, CBO, BPC], f32)
                ATe = bigp.tile([65, CB, BPC], f16)
                ATo = bigp.tile([64, CBO, BPC], f16)
                for c0, c1, c1o in ((0, 8, 8), (8, 16, 15)):
                    nc.tensor.matmul(eSq[:, c0:c1], wsq[:], sqe[:, c0:c1],
                                     start=True, stop=True)
                    nc.tensor.matmul(eSm[:, c0:c1], wsm[:], xTe[:, c0:c1],
                                     start=True, stop=True)
                    nc.tensor.matmul(oSq[:, c0:c1o], wsq[:, 0:64],
                                     sqo[:, c0:c1o], start=True, stop=True)
                    nc.tensor.matmul(oSm[:, c0:c1o], wsm[:, 0:64],
                                     xTo[:, c0:c1o], start=True, stop=True)
                    nc.scalar.activation(swE[:, c0:c1], eSm[:, c0:c1],
                                         AF.Square, scale=0.125)
                    nc.scalar.activation(swO[:, c0:c1o], oSm[:, c0:c1o],
                                         AF.Square, scale=0.125)
                    nc.vector.tensor_sub(ATe[:, c0:c1], eSq[:, c0:c1],
                                         swE[:, c0:c1])
                    nc.vector.tensor_sub(ATo[:, c0:c1o], oSq[:, c0:c1o],
                                         swO[:, c0:c1o])

            # ---- combined-tile assembly (HWDGE sync/scalar, or SWDGE),
            # chunked by transpose half so CT1's first columns are ready
            # as soon as the first x chunks land.
            ctq1 = nc.gpsimd if _FLAGS["ct_sw"] else nc.sync
            ctq2 = nc.gpsimd if _FLAGS["ct_sw"] else nc.scalar
            # x rows (even c 0:8 from chunk 1 first, then the rest)
            ctq1.dma_start(CT1[0:96, 0, 0:8], xTe[0:96, 0:8])
            ctq1.dma_start(CT1[0:96, 1, 0:8], xTo[0:96, 0:8])
            ctq1.dma_start(CT1[0:96, 0, 8:16], xTe[0:96, 8:16])
            ctq1.dma_start(CT1[0:96, 1, 8:15], xTo[0:96, 8:15])
            ctq1.dma_start(CT2[32:128, 0], xTe[32:128])
            ctq1.dma_start(CT2[32:128, 1, 0:15], xTo[32:128])
            # A rows (chunked likewise; CT2's A rows are needed ~mid-sweep)
            ctq2.dma_start(CT1[96:128, 0, 0:8], ATe[0:32, 0:8])
            ctq2.dma_start(CT1[96:128, 1, 0:8], ATo[0:32, 0:8])
            ctq2.dma_start(CT1[96:128, 0, 8:16], ATe[0:32, 8:16])
            ctq2.dma_start(CT1[96:128, 1, 8:15], ATo[0:32, 8:15])
            ctq2.dma_start(CT2[0:32, 0], ATe[32:64])
            ctq2.dma_start(CT2[0:32, 1, 0:15], ATo[32:64])
            # (the (1,15) garbage slot is never read: the odd-half second
            # matmul is N=448, so no memsets are needed)

            # ---- main sweep.  Default: per r, 4 matmuls into two 2-bank
            # PSUM tiles; 'a': two ACT casts + one DVE TT fold; 'b': two
            # DVE TT-max straight off PSUM into a (j,b) accumulator.
            with tc.tile_pool(name=f"psM{_it}", bufs=4, space="PSUM") as psM, \
                 tc.tile_pool(name=f"drain{_it}", bufs=4) as drp:
                # edge window p=1984 first: rows 64..127 of the j=30
                # column; its drain is one cheap DVE copy.
                pe = psM.tile([S, CB, BPC], f32, tag="ph")
                nc.tensor.matmul(pe[:, 0, :], Wn[:, 64, :],
                                 CT2[:, 0, 15], start=True, stop=False)
                # A[b,1984] sits at ATe[64, 15, b]; nmask is -1 at row 64
                nc.tensor.matmul(pe[:, 0, :],
                                 nmask[0:65].broadcast_to([65, S]),
                                 ATe[:, 15], start=False, stop=True)
                nc.vector.tensor_copy(STR[:, :, 0], pe[:, 0, :])

                na = 0
                nb = 0
                for r in range(64):
                    W = Wn[:, r, :]
                    CT = CT1 if r < 32 else CT2
                    path = PATHS[r] if _FLAGS["drain"] else "n"
                    sbm = (drp.tile([S, J - 1, BPC], f16, name="sbm",
                                    tag="sm") if path == "a" else None)
                    for h in range(2):
                        ph = psM.tile([S, CB, BPC], f32, tag="ph")
                        j0 = 16 * h
                        nv = 16 if h == 0 else 15   # drop the j=31 garbage
                        if _FLAGS["mains"]:
                            nc.tensor.matmul(ph[:, 0:8], W, CT[:, h, 0:8],
                                             start=True, stop=True)
                            nc.tensor.matmul(ph[:, 8:nv], W, CT[:, h, 8:nv],
                                             start=True, stop=True)
                            if _FLAGS["pe_x2"]:
                                nc.tensor.matmul(ph[:, 0:8], W,
                                                 CT[:, h, 0:8],
                                                 start=True, stop=True)
                                nc.tensor.matmul(ph[:, 8:nv], W,
                                                 CT[:, h, 8:nv],
                                                 start=True, stop=True)
                        if path == "a":
                            if _FLAGS["cast4"]:
                                nc.scalar.mul(sbm[:, j0:j0 + 8],
                                              ph[:, 0:8], 1.0)
                                nc.scalar.mul(sbm[:, j0 + 8:j0 + nv],
                                              ph[:, 8:nv], 1.0)
                            else:
                                nc.scalar.mul(sbm[:, j0:j0 + nv],
                                              ph[:, 0:nv], 1.0)
                        elif path == "b":
                            bi = nb % len(baccs)
                            bt = baccs[bi]
                            if not binit[bi][h]:
                                # first touch initialises by PSUM copy
                                nc.vector.tensor_copy(bt[:, j0:j0 + nv],
                                                      ph[:, 0:nv])
                                binit[bi][h] = True
                            else:
                                nc.vector.tensor_tensor(
                                    bt[:, j0:j0 + nv], ph[:, 0:nv],
                                    bt[:, j0:j0 + nv], OP.max)
                    if path == "a":
                        if maccs:
                            idx = na % len(maccs)
                            if can_early and na >= n_a - 2:
                                idx = (na + 1) % 2
                            m16 = maccs[idx]
                            if na < len(maccs):
                                nc.vector.tensor_copy(m16[:], sbm[:])
                            else:
                                nc.vector.tensor_tensor(m16[:], sbm[:],
                                                        m16[:], OP.max)
                            if can_early and na == n_a - 2:
                                # macc1 is final; merge into macc0 while
                                # the last cast runs
                                nc.vector.tensor_tensor(
                                    maccs[0][:], maccs[0][:], maccs[1][:],
                                    OP.max)
                        na += 1
                    elif path == "b":
                        nb += 1
                    if can_early and r == 54:
                        # all 'b' r's done by r=53: merge baccs in slack
                        nc.vector.tensor_tensor(baccs[0][:], baccs[0][:],
                                                baccs[1][:], OP.max)
                    if can_early and r == 56 and maccs:
                        nc.vector.tensor_tensor(maccs[0][:], maccs[0][:],
                                                baccs[0][:], OP.max)

                # ---- finish
                r16 = constp.tile([S, BPC], f32)
                if can_early and maccs:
                    # in-place TT tree over j (2x_1P fp16) instead of the
                    # 1x strided tensor_reduce: 31 -> 15(+slot15) -> 8 ->
                    # 4 -> 2 -> 1, then merge the edge strip.
                    m = maccs[0]
                    nc.vector.tensor_tensor(m[:, 0:15], m[:, 0:15],
                                            m[:, 16:31], OP.max)
                    nc.vector.tensor_tensor(m[:, 0:8], m[:, 0:8],
                                            m[:, 8:16], OP.max)
                    nc.vector.tensor_tensor(m[:, 0:4], m[:, 4:8],
                                            m[:, 0:4], OP.max)
                    nc.vector.tensor_tensor(m[:, 0:2], m[:, 2:4],
                                            m[:, 0:2], OP.max)
                    nc.vector.tensor_tensor(m[:, 0:1], m[:, 1:2],
                                            m[:, 0:1], OP.max)
                    nc.vector.tensor_tensor(r16[:], m[:, 0, :],
                                            STR[:, :, 0], OP.max)
                else:
                    folds = list(maccs) + list(baccs)
                    if folds:
                        for t in folds[1:]:
                            nc.vector.tensor_tensor(
                                folds[0][:], folds[0][:], t[:], OP.max)
                        nc.vector.tensor_reduce(
                            r16[:], folds[0][:].rearrange("p j b -> p b j"),
                            axis=AX, op=OP.max)
                        nc.vector.tensor_tensor(r16[:], r16[:],
                                                STR[:, :, 0], OP.max)
                    else:
                        nc.vector.tensor_copy(r16[:], STR[:, :, 0])
                # d = sqrt(relu(s2 - v)):  (v - s2) clamped <= 0, Sqrt(-x)
                nc.vector.tensor_scalar(r16[:], r16[:], s2[:], 0.0,
                                        OP.subtract, OP.min)
                res = constp.tile([S, BPC], f32)
                nc.scalar.activation(res[:], r16[:], AF.Sqrt, scale=-1.0)

            with tc.tile_pool(name=f"psC{_it}", bufs=1, space="PSUM") as psC:
                po = psC.tile([BPC, S], f32)
                nc.tensor.transpose(po[:], res[:], ident[:])
                outsb = constp.tile([BPC, S], f32)
                nc.scalar.mul(outsb[:], po[:], 1.0)
                nc.sync.dma_start(out_d[:], outsb[:])


def _wn_np(sh):
    # sh: (S, L) float32 -> Wn (128, 65, S) fp16:
    #   rows [r, r+64) of slot r hold +2 s~[s, k-r]; indicator row -1.
    st = 2.0 * (sh - sh.mean(axis=1, keepdims=True))       # (S, L)
    wn = np.zeros((128, 65, S), dtype=np.float32)
    for r in range(65):
        wn[r:r + 64, r, :] = st.T
        if r < 32:
            wn[96 + r, r, :] = -1.0
        elif r < 64:
            wn[r - 32, r, :] = -1.0
    return wn.astype(np.float16)


def _wins_np():
    # ones-window weights: wsq[k, m] = 1 for k in [m, m+64)
    w = np.zeros((128, 65), dtype=np.float16)
    for m in range(65):
        w[m:m + 64, m] = 1.0
    return w


def _get_nc():
    if "nc" not in _STATE:
        from concourse import bacc
        nc = bacc.Bacc("TRN2", target_bir_lowering=False, debug=False,
                       num_devices=NCORES)
        _build(nc)
        nc.compile()
        _STATE["nc"] = nc
    return _STATE["nc"]


def _in_maps(x, shapelets):
    x = np.ascontiguousarray(np.asarray(x, dtype=np.float32)).reshape(B, T)
    sh = np.ascontiguousarray(
        np.asarray(shapelets, dtype=np.float32)).reshape(S, L)
    wn = _wn_np(sh)
    wsq = _wins_np()
    nmask = np.zeros((128, 1), dtype=np.float16)
    nmask[64, 0] = -1.0
    s2v = (sh * sh).sum(axis=1, dtype=np.float32).reshape(S, 1)
    ident = np.eye(128, dtype=np.float32)
    return [{"x_shard": x[i * BPC:(i + 1) * BPC], "wn": wn, "wsq": wsq,
             "wsm": wsq, "nmask": nmask, "s2v": s2v, "ident": ident}
            for i in range(NCORES)]


def kernel(x, shapelets):
    from concourse.bass_utils import run_bass_kernel_spmd
    nc = _get_nc()
    res = run_bass_kernel_spmd(nc, _in_maps(x, shapelets),
                               list(range(NCORES))).results
    return np.concatenate([res[i]["out"] for i in range(NCORES)], axis=0)


if __name__ == "__main__":
    rng = np.random.default_rng(0)
    x = rng.standard_normal((B, C, T)).astype(np.float32)
    sh = rng.standard_normal((S, C, L)).astype(np.float32)
    out = kernel(x, sh)
    print("out", out.shape, out.dtype, float(out.min()), float(out.max()))
